# revision 21
# baseline (speedup 1.0000x reference)
"""Trainium2 Bass kernel for GQA attention with RoPE (B=2, S=1024, HID=2048,
16 q heads / 4 kv heads, head dim 128, causal).

Sharding: 8 cores = 2 batches x 4 kv-head groups. Core c = b*4 + g handles
batch b and kv head g (query heads 4g..4g+3). Each core computes a partial
output y_part = attn_heads @ wo_shard; the host sums the 4 partials per batch.

Per-core dataflow (matmuls fp32r, moving free dim >= 256):
  Phase A (per 128-row chunk g, software-pipelined 2 deep):
    x chunk --PE transpose--> xT --mm--> q, [k|v] (natural); RoPE on DVE;
    PE transpose q_rope/k_rope -> persistent qT[d,h,s], kT[d,s], v[s,d].
  Phase B/C (per 256-col macro tile, heads pipelined one deep):
    scoresT[sk,sq] = kT_chunk.T @ qT ; expS = exp(scale*s + mask)  (ACT)
    denom_rep = ones.T @ expS ; U^T = v.T-free @ expS   (PE, accumulated)
    rec = exp(-ln(denom))  (ACT) ; uT = U^T * rec  (DVE, fused with copy)
    y = sum_h uT_h.T @ wo_h  (PE) -> SBUF -> DRAM
"""

import sys

import numpy as np

for _p in ("/opt/trn_rl_repo", "/root/.axon_site/_ro/trn_rl_repo"):
    if _p not in sys.path:
        sys.path.append(_p)

from contextlib import ExitStack

import concourse.bass as bass
import concourse.mybir as mybir
from concourse import bacc
from concourse.masks import make_identity
from concourse.tile import TileContext

P = 128           # partitions / head dim / seq chunk
S = 1024          # sequence length
HID = 2048        # model dim
NH = 4            # query heads per core
D = 128           # head dim
TQ = 256          # query macro-tile (matmul moving free dim)
NT = S // TQ      # 4 macro tiles
KC = HID // P     # 16 contraction chunks
NSK = S // P      # 8 key chunks
NG = S // P       # 8 row chunks
F32 = mybir.dt.float32
F32R = mybir.dt.float32r
SCALE = 1.0 / float(np.sqrt(D))
NEG = -30000.0
AL = mybir.AluOpType
AF = mybir.ActivationFunctionType

N_CORES = 8
B = 2
N_KV = 4


def build_nc():
    nc = bacc.Bacc("TRN2", target_bir_lowering=False, debug=False)
    x_d = nc.declare_dram_parameter("x", [S, HID], F32, isOutput=False)
    cos_d = nc.declare_dram_parameter("cos", [S, D], F32, isOutput=False)
    sin_d = nc.declare_dram_parameter("sin", [S, D], F32, isOutput=False)
    wq_d = nc.declare_dram_parameter("wq", [HID, NH * D], F32R, isOutput=False)
    wk_d = nc.declare_dram_parameter("wk", [HID, D], F32R, isOutput=False)
    wv_d = nc.declare_dram_parameter("wv", [HID, D], F32R, isOutput=False)
    wo_d = nc.declare_dram_parameter("wo", [NH * D, HID], F32R, isOutput=False)
    out_d = nc.declare_dram_parameter("out", [S, HID], F32, isOutput=True)

    with TileContext(nc) as tc, ExitStack() as ctx:
        consts = ctx.enter_context(tc.tile_pool(name="consts", bufs=1))
        wpool = ctx.enter_context(tc.tile_pool(name="wpool", bufs=1))
        persist = ctx.enter_context(tc.tile_pool(name="persist", bufs=1))

        # ---- constants ----
        ident = consts.tile([P, P], F32, tag="ident")
        make_identity(nc, ident)
        ones_f32 = consts.tile([P, P], F32, tag="ones_f32")
        nc.vector.memset(ones_f32, 1.0)
        ones = consts.tile([P, P], F32R, tag="ones")
        nc.vector.tensor_copy(ones, ones_f32)
        # Causal masks for the two diagonal-straddling chunk positions.
        m12 = consts.tile([P, 2 * TQ], F32, tag="m12")
        nc.gpsimd.memset(m12, 0.0)
        nc.gpsimd.affine_select(
            out=m12[:, 0:TQ], in_=m12[:, 0:TQ], compare_op=AL.is_ge, fill=NEG,
            base=0, pattern=[[1, TQ]], channel_multiplier=-1,
        )
        nc.gpsimd.affine_select(
            out=m12[:, TQ : 2 * TQ], in_=m12[:, TQ : 2 * TQ],
            compare_op=AL.is_ge, fill=NEG,
            base=-P, pattern=[[1, TQ]], channel_multiplier=-1,
        )

        # ---- weights (partition-chunked layouts), interleaved with x loads ----
        wq_sb = wpool.tile([P, KC, NH * D], F32R, tag="wq")
        wq_r = wq_d[:].rearrange("(c p) n -> p c n", p=P)
        wkv_sb = wpool.tile([P, KC, 2 * D], F32R, tag="wkv")
        wo_sb = wpool.tile([P, NH, HID], F32R, tag="wo")
        wo_r = wo_d[:].rearrange("(h p) n -> p h n", p=P)
        cos_sb = wpool.tile([P, NG, D], F32, tag="cos")
        sin_sb = wpool.tile([P, NG, D], F32, tag="sin")

        # persistent transposed activations
        qT_all = persist.tile([P, NH, S], F32R, tag="qT")   # [d, h, sq]
        kT = persist.tile([P, S], F32R, tag="kT")           # [d, sk]
        vv = persist.tile([P, NSK, D], F32R, tag="vv")      # v natural [sk, d]

        H2 = D // 2

        def rope(dst, src, g, tmp_tag, wk):
            """dst = src*cos + rotate_half(src)*sin, natural layout [P, D]."""
            cos_g = cos_sb[:, g, :]
            sin_g = sin_sb[:, g, :]
            tmp = wk.tile([P, D], F32, tag=tmp_tag)
            nc.vector.scalar_tensor_tensor(
                out=tmp[:, 0:H2], in0=src[:, H2:D], scalar=-1.0,
                in1=sin_g[:, 0:H2], op0=AL.mult, op1=AL.mult,
            )
            nc.vector.tensor_tensor(
                out=tmp[:, H2:D], in0=src[:, 0:H2], in1=sin_g[:, H2:D], op=AL.mult
            )
            nc.vector.tensor_tensor(out=dst, in0=src, in1=cos_g, op=AL.mult)
            nc.vector.tensor_tensor(out=dst, in0=dst, in1=tmp, op=AL.add)

        # ================= Phase A: projections =================
        with tc.tile_pool(name="pA_sb", bufs=2) as pa, \
             tc.tile_pool(name="pA_tp", bufs=3, space="PSUM") as ps_tp, \
             tc.tile_pool(name="pA_warm", bufs=1, space="PSUM") as ps_warm, \
             tc.tile_pool(name="pA_qkv", bufs=2, space="PSUM") as ps_qkv:

            # dummy matmuls to lift the PE HAM clock gate to 8/8 while the
            # first x/weight DMAs are still in flight
            warm_ps = ps_warm.tile([P, P], F32, tag="warm")
            for _ in range(26):
                nc.tensor.matmul(warm_ps, ones, ones, start=True, stop=True)

            x_tiles = [None] * NG
            pend = [None] * NG  # g -> (q_ps3, kv_ps, xT)

            def emit_xdma(g):
                x_nat = pa.tile([P, HID], F32, tag="xnat", bufs=3)
                nc.sync.dma_start(out=x_nat, in_=x_d[g * P : (g + 1) * P, :])
                x_tiles[g] = x_nat

            # DMA order: x0, wq(2), wkv, x1, cos, sin, x2.., wo(4) trailing
            emit_xdma(0)
            nc.sync.dma_start(out=wq_sb[:, 0:4, :], in_=wq_r[:, 0:4, :])
            nc.sync.dma_start(out=wq_sb[:, 4:8, :], in_=wq_r[:, 4:8, :])
            emit_xdma(1)
            nc.sync.dma_start(out=wq_sb[:, 8:12, :], in_=wq_r[:, 8:12, :])
            nc.sync.dma_start(out=wq_sb[:, 12:16, :], in_=wq_r[:, 12:16, :])
            nc.sync.dma_start(
                out=wkv_sb[:, :, 0:D], in_=wk_d[:].rearrange("(c p) n -> p c n", p=P)
            )
            nc.sync.dma_start(
                out=wkv_sb[:, :, D : 2 * D],
                in_=wv_d[:].rearrange("(c p) n -> p c n", p=P),
            )
            emit_xdma(2)
            nc.sync.dma_start(
                out=cos_sb, in_=cos_d[:].rearrange("(c p) d -> p c d", p=P)
            )
            nc.sync.dma_start(
                out=sin_sb, in_=sin_d[:].rearrange("(c p) d -> p c d", p=P)
            )
            wo_next = [0]

            def emit_wo_dma():
                h = wo_next[0]
                if h < NH:
                    nc.sync.dma_start(out=wo_sb[:, h, :], in_=wo_r[:, h, :])
                    wo_next[0] += 1

            def transposes(g):
                """x chunk -> xT (PE transpose + DVE cast-copy)."""
                x_nat = x_tiles[g]
                xT = pa.tile([P, KC, P], F32R, tag="xT", bufs=3)
                xT_flat = xT.rearrange("p c d -> p (c d)")
                for kb in range(KC // 4):
                    tp_ps = ps_tp.tile([P, 4 * P], F32, tag="tp")
                    for j in range(4):
                        k = 4 * kb + j
                        nc.tensor.transpose(
                            tp_ps[:, j * P : (j + 1) * P],
                            x_nat[:, k * P : (k + 1) * P],
                            ident,
                        )
                    if kb % 2 == 0:
                        nc.vector.tensor_copy(
                            xT_flat[:, kb * 4 * P : (kb + 1) * 4 * P], tp_ps
                        )
                    else:
                        nc.scalar.activation(
                            out=xT_flat[:, kb * 4 * P : (kb + 1) * 4 * P], in_=tp_ps,
                            func=AF.Copy,
                        )
                return xT

            def proj(g, xT):
                """q and kv projections for chunk g (PE, accumulating)."""
                qkv_ps = ps_qkv.tile([P, NH * D + 2 * D], F32, tag="qkv")
                q_ps = qkv_ps[:, 0 : NH * D]
                kv_ps = qkv_ps[:, NH * D : NH * D + 2 * D]
                for k in range(KC):
                    nc.tensor.matmul(
                        q_ps, xT[:, k, :], wq_sb[:, k, :],
                        start=(k == 0), stop=(k == KC - 1),
                    )
                for k in range(KC):
                    nc.tensor.matmul(
                        kv_ps, xT[:, k, :], wkv_sb[:, k, :],
                        start=(k == 0), stop=(k == KC - 1),
                    )
                return qkv_ps

            def rope_stage(g, qkv_ps):
                """RoPE on q heads + k (DVE), v copy-out."""
                q3 = qkv_ps[:, 0 : NH * D].rearrange("p (h d) -> p h d", h=NH)
                kv_ps = qkv_ps[:, NH * D : NH * D + 2 * D]
                q_rope = pa.tile([P, NH, D], F32, tag="qrope")
                for h in range(NH):
                    rope(q_rope[:, h, :], q3[:, h, :], g, "tmq", pa)
                k_rope = pa.tile([P, D], F32, tag="krope")
                rope(k_rope, kv_ps[:, 0:D], g, "tmk", pa)
                nc.vector.tensor_copy(vv[:, g, :], kv_ps[:, D : 2 * D])
                return q_rope, k_rope

            def rope_transpose(g, q_rope, k_rope):
                """Transpose RoPE'd q/k into persistent qT_all / kT."""
                tq_ps = ps_tp.tile([P, 4 * P], F32, tag="tp")
                for h in range(NH):
                    nc.tensor.transpose(
                        tq_ps[:, h * P : (h + 1) * P], q_rope[:, h, :], ident
                    )
                nc.vector.tensor_copy(
                    qT_all[:, :, g * P : (g + 1) * P],
                    tq_ps.rearrange("p (h d) -> p h d", h=NH),
                )
                tk_ps = ps_tp.tile([P, 4 * P], F32, tag="tp")
                nc.tensor.transpose(tk_ps[:, 0:P], k_rope, ident)
                nc.vector.tensor_copy(kT[:, g * P : (g + 1) * P], tk_ps[:, 0:P])

            # 2-deep software pipeline over chunks
            ropes = [None] * NG
            for g in range(NG + 2):
                if g >= 2:
                    gg = g - 2
                    sc = nc.named_scope(f"rope_{gg}"); sc.__enter__()
                    ropes[gg] = rope_stage(gg, pend[gg][1])
                    sc.__exit__(None, None, None)
                if g < NG:
                    if g + 3 < NG:
                        emit_xdma(g + 3)
                    if g >= 4:
                        emit_wo_dma()
                    sc = nc.named_scope(f"tp_{g}"); sc.__enter__()
                    xT = transposes(g)
                    sc.__exit__(None, None, None)
                    pend[g] = [xT, None, None]
                if g >= 1 and g - 1 < NG:
                    gg = g - 1
                    sc = nc.named_scope(f"proj_{gg}"); sc.__enter__()
                    qkv_ps = proj(gg, pend[gg][0])
                    sc.__exit__(None, None, None)
                    pend[gg][1] = qkv_ps
                if g >= 2:
                    gg = g - 2
                    sc = nc.named_scope(f"ropeT_{gg}"); sc.__enter__()
                    rope_transpose(gg, *ropes[gg])
                    sc.__exit__(None, None, None)
                    pend[gg] = None
            emit_wo_dma()
            emit_wo_dma()
            emit_wo_dma()
            emit_wo_dma()

        # ================= Phase B/C: attention + output =================
        with tc.tile_pool(name="pB_sb", bufs=2) as pb, \
             tc.tile_pool(name="pB_s", bufs=3, space="PSUM") as ps_s, \
             tc.tile_pool(name="pB_ud", bufs=2, space="PSUM") as ps_ud, \
             tc.tile_pool(name="pB_y", bufs=2, space="PSUM") as ps_y:

            def scores_head(t, h):
                """scoresT + exp for head h of macro tile t -> expst tile.

                Chunk pairs share one full PSUM bank so the causal mask is a
                single DVE add and exp is one ACT op per pair."""
                qT_h = qT_all[:, h, t * TQ : (t + 1) * TQ]
                expst = pb.tile([P, NSK, TQ], F32R, tag="expst")
                expst_flat = expst.rearrange("p c f -> p (c f)")
                for pi in range(t + 1):
                    s_ps = ps_s.tile([P, 2 * TQ], F32, tag="s")
                    for half in range(2):
                        ik = 2 * pi + half
                        nc.tensor.matmul(
                            s_ps[:, half * TQ : (half + 1) * TQ],
                            kT[:, ik * P : (ik + 1) * P], qT_h,
                            start=True, stop=True,
                        )
                    if pi == t:
                        nc.vector.tensor_tensor(out=s_ps, in0=s_ps, in1=m12, op=AL.add)
                    nc.scalar.activation(
                        out=expst_flat[:, pi * 2 * TQ : (pi + 1) * 2 * TQ],
                        in_=s_ps, func=AF.Exp, scale=SCALE,
                    )
                return expst

            def dnpv_head(t, h, expst, uT_t):
                """denominator + PV matmuls, then normalize into uT_t (DVE)."""
                nsk = 2 * (t + 1)
                ud_ps = ps_ud.tile([P, 2 * TQ], F32, tag="ud")
                u_ps = ud_ps[:, 0:TQ]
                den_ps = ud_ps[:, TQ : 2 * TQ]
                for ik in range(nsk):
                    nc.tensor.matmul(
                        den_ps, ones, expst[:, ik, :],
                        start=(ik == 0), stop=(ik == nsk - 1),
                    )
                for ik in range(nsk):
                    nc.tensor.matmul(
                        u_ps, vv[:, ik, :], expst[:, ik, :],
                        start=(ik == 0), stop=(ik == nsk - 1),
                    )
                rec = pb.tile([P, TQ], F32, tag="rec")
                nc.vector.reciprocal(rec, den_ps)
                nc.vector.tensor_tensor(
                    out=uT_t[:, h, :], in0=u_ps, in1=rec, op=AL.mult
                )

            def wo_stage(t, uT_t):
                for sub in range(2):
                    g = 2 * t + sub
                    for n in range(HID // 512):
                        y_ps = ps_y.tile([P, 512], F32, tag="y")
                        for h in range(NH):
                            nc.tensor.matmul(
                                y_ps,
                                uT_t[:, h, sub * P : (sub + 1) * P],
                                wo_sb[:, h, n * 512 : (n + 1) * 512],
                                start=(h == 0), stop=(h == NH - 1),
                            )
                        y_sb = pb.tile([P, 512], F32, tag="ysb", bufs=3)
                        nc.vector.tensor_copy(y_sb, y_ps)
                        nc.gpsimd.dma_start(
                            out=out_d[g * P : (g + 1) * P, n * 512 : (n + 1) * 512],
                            in_=y_sb,
                        )

            # heads pipelined one deep; wo lags two score-steps so the
            # last head's DVE normalize is off the PE critical path
            steps = [(t, h) for t in range(NT) for h in range(NH)]
            uts = {}
            for i in range(len(steps) + 2):
                if i < len(steps):
                    t, h = steps[i]
                    if h == 0:
                        uts[t] = pb.tile([P, NH, TQ], F32R, tag="uT", name=f"uT{t}")
                    sc = nc.named_scope(f"sc_{t}_{h}"); sc.__enter__()
                    uts[(t, h)] = scores_head(t, h)
                    sc.__exit__(None, None, None)
                if 1 <= i < len(steps) + 1:
                    t, h = steps[i - 1]
                    sc = nc.named_scope(f"dnpv_{t}_{h}"); sc.__enter__()
                    dnpv_head(t, h, uts.pop((t, h)), uts[t])
                    sc.__exit__(None, None, None)
                if i >= 2 and (i - 2) % NH == NH - 1:
                    t = (i - 2) // NH
                    sc = nc.named_scope(f"wo_{t}"); sc.__enter__()
                    wo_stage(t, uts.pop(t))
                    sc.__exit__(None, None, None)

    nc.compile()
    return nc


def shard_inputs(x, cos, sin, wq, wk, wv, wo):
    """Build per-core input maps: core = b*4 + g."""
    in_maps = []
    for c in range(N_CORES):
        b, g = divmod(c, N_KV)
        in_maps.append(
            {
                "x": np.ascontiguousarray(x[b]),
                "cos": np.ascontiguousarray(cos),
                "sin": np.ascontiguousarray(sin),
                "wq": np.ascontiguousarray(wq[:, g * NH * D : (g + 1) * NH * D]),
                "wk": np.ascontiguousarray(wk[:, g * D : (g + 1) * D]),
                "wv": np.ascontiguousarray(wv[:, g * D : (g + 1) * D]),
                "wo": np.ascontiguousarray(wo[g * NH * D : (g + 1) * NH * D, :]),
            }
        )
    return in_maps


_NC_CACHE = {}


def get_nc():
    if "nc" not in _NC_CACHE:
        _NC_CACHE["nc"] = build_nc()
    return _NC_CACHE["nc"]


def kernel(x, cos, sin, wq, wk, wv, wo, _trace=False):
    from concourse.bass_utils import run_bass_kernel_spmd

    x = np.asarray(x, dtype=np.float32)
    cos = np.asarray(cos, dtype=np.float32)
    sin = np.asarray(sin, dtype=np.float32)
    wq = np.asarray(wq, dtype=np.float32)
    wk = np.asarray(wk, dtype=np.float32)
    wv = np.asarray(wv, dtype=np.float32)
    wo = np.asarray(wo, dtype=np.float32)

    nc = get_nc()
    in_maps = shard_inputs(x, cos, sin, wq, wk, wv, wo)
    res = run_bass_kernel_spmd(nc, in_maps, list(range(N_CORES)), trace=_trace)
    parts = [np.asarray(res.results[c]["out"], dtype=np.float32) for c in range(N_CORES)]
    y = np.stack(
        [sum(parts[b * N_KV + g] for g in range(N_KV)) for b in range(B)], axis=0
    )
    if _trace:
        kernel.last_result = res
    return y


# revision 23
# speedup vs baseline: 1.0153x; 1.0153x over previous
"""Trainium2 Bass kernel for GQA attention with RoPE (B=2, S=1024, HID=2048,
16 q heads / 4 kv heads, head dim 128, causal).

Sharding: 8 cores = 2 batches x 4 kv-head groups. Core c = b*4 + g handles
batch b and kv head g (query heads 4g..4g+3). Each core computes a partial
output y_part = attn_heads @ wo_shard; the host sums the 4 partials per batch.

Per-core dataflow (matmuls fp32r, moving free dim >= 256):
  Phase A (per 128-row chunk g, software-pipelined 2 deep):
    x chunk --PE transpose--> xT --mm--> q, [k|v] (natural); RoPE on DVE;
    PE transpose q_rope/k_rope -> persistent qT[d,h,s], kT[d,s], v[s,d].
  Phase B/C (per 256-col macro tile, heads pipelined one deep):
    scoresT[sk,sq] = kT_chunk.T @ qT ; expS = exp(scale*s + mask)  (ACT)
    denom_rep = ones.T @ expS ; U^T = v.T-free @ expS   (PE, accumulated)
    rec = exp(-ln(denom))  (ACT) ; uT = U^T * rec  (DVE, fused with copy)
    y = sum_h uT_h.T @ wo_h  (PE) -> SBUF -> DRAM
"""

import sys

import numpy as np

for _p in ("/opt/trn_rl_repo", "/root/.axon_site/_ro/trn_rl_repo"):
    if _p not in sys.path:
        sys.path.append(_p)

from contextlib import ExitStack

import concourse.bass as bass
import concourse.mybir as mybir
from concourse import bacc
from concourse.masks import make_identity
from concourse.tile import TileContext

P = 128           # partitions / head dim / seq chunk
S = 1024          # sequence length
HID = 2048        # model dim
NH = 4            # query heads per core
D = 128           # head dim
TQ = 256          # query macro-tile (matmul moving free dim)
NT = S // TQ      # 4 macro tiles
KC = HID // P     # 16 contraction chunks
NSK = S // P      # 8 key chunks
NG = S // P       # 8 row chunks
F32 = mybir.dt.float32
F32R = mybir.dt.float32r
SCALE = 1.0 / float(np.sqrt(D))
NEG = -30000.0
AL = mybir.AluOpType
AF = mybir.ActivationFunctionType

N_CORES = 8
B = 2
N_KV = 4


def build_nc():
    nc = bacc.Bacc("TRN2", target_bir_lowering=False, debug=False)
    x_d = nc.declare_dram_parameter("x", [S, HID], F32R, isOutput=False)
    cos_d = nc.declare_dram_parameter("cos", [S, D], F32, isOutput=False)
    sin_d = nc.declare_dram_parameter("sin", [S, D], F32, isOutput=False)
    wq_d = nc.declare_dram_parameter("wq", [HID, NH * D], F32R, isOutput=False)
    wk_d = nc.declare_dram_parameter("wk", [HID, D], F32R, isOutput=False)
    wv_d = nc.declare_dram_parameter("wv", [HID, D], F32R, isOutput=False)
    wo_d = nc.declare_dram_parameter("wo", [NH * D, HID], F32R, isOutput=False)
    out_d = nc.declare_dram_parameter("out", [S, HID], F32, isOutput=True)

    with TileContext(nc) as tc, ExitStack() as ctx:
        consts = ctx.enter_context(tc.tile_pool(name="consts", bufs=1))
        wpool = ctx.enter_context(tc.tile_pool(name="wpool", bufs=1))
        persist = ctx.enter_context(tc.tile_pool(name="persist", bufs=1))

        # ---- constants ----
        ident_f32 = consts.tile([P, P], F32, tag="ident_f32")
        make_identity(nc, ident_f32)
        ident = consts.tile([P, P], F32R, tag="ident")
        nc.vector.tensor_copy(ident, ident_f32)
        ones_f32 = consts.tile([P, P], F32, tag="ones_f32")
        nc.vector.memset(ones_f32, 1.0)
        ones = consts.tile([P, P], F32R, tag="ones")
        nc.vector.tensor_copy(ones, ones_f32)
        # Causal masks for the two diagonal-straddling chunk positions.
        m12 = consts.tile([P, 2 * TQ], F32, tag="m12")
        nc.gpsimd.memset(m12, 0.0)
        nc.gpsimd.affine_select(
            out=m12[:, 0:TQ], in_=m12[:, 0:TQ], compare_op=AL.is_ge, fill=NEG,
            base=0, pattern=[[1, TQ]], channel_multiplier=-1,
        )
        nc.gpsimd.affine_select(
            out=m12[:, TQ : 2 * TQ], in_=m12[:, TQ : 2 * TQ],
            compare_op=AL.is_ge, fill=NEG,
            base=-P, pattern=[[1, TQ]], channel_multiplier=-1,
        )

        # ---- weights (partition-chunked layouts), interleaved with x loads ----
        wq_sb = wpool.tile([P, KC, NH * D], F32R, tag="wq")
        wq_r = wq_d[:].rearrange("(c p) n -> p c n", p=P)
        wkv_sb = wpool.tile([P, KC, 2 * D], F32R, tag="wkv")
        wo_sb = wpool.tile([P, NH, HID], F32R, tag="wo")
        wo_r = wo_d[:].rearrange("(h p) n -> p h n", p=P)
        cos_sb = wpool.tile([P, NG, D], F32, tag="cos")
        sin_sb = wpool.tile([P, NG, D], F32, tag="sin")

        # persistent transposed activations
        qT_all = persist.tile([P, NH, S], F32R, tag="qT")   # [d, h, sq]
        kT = persist.tile([P, S], F32R, tag="kT")           # [d, sk]
        vv = persist.tile([P, NSK, D], F32R, tag="vv")      # v natural [sk, d]

        H2 = D // 2

        def rope(dst, src, g, tmp_tag, wk):
            """dst = src*cos + rotate_half(src)*sin, natural layout [P, D]."""
            cos_g = cos_sb[:, g, :]
            sin_g = sin_sb[:, g, :]
            tmp = wk.tile([P, D], F32, tag=tmp_tag)
            nc.vector.scalar_tensor_tensor(
                out=tmp[:, 0:H2], in0=src[:, H2:D], scalar=-1.0,
                in1=sin_g[:, 0:H2], op0=AL.mult, op1=AL.mult,
            )
            nc.vector.tensor_tensor(
                out=tmp[:, H2:D], in0=src[:, 0:H2], in1=sin_g[:, H2:D], op=AL.mult
            )
            nc.vector.tensor_tensor(out=dst, in0=src, in1=cos_g, op=AL.mult)
            nc.vector.tensor_tensor(out=dst, in0=dst, in1=tmp, op=AL.add)

        # ================= Phase A: projections =================
        with tc.tile_pool(name="pA_sb", bufs=2) as pa, \
             tc.tile_pool(name="pA_tp", bufs=3, space="PSUM") as ps_tp, \
             tc.tile_pool(name="pA_warm", bufs=1, space="PSUM") as ps_warm, \
             tc.tile_pool(name="pA_qkv", bufs=2, space="PSUM") as ps_qkv:

            # dummy matmuls to lift the PE HAM clock gate to 8/8 while the
            # first x/weight DMAs are still in flight
            warm_ps = ps_warm.tile([P, P], F32, tag="warm")
            for _ in range(26):
                nc.tensor.matmul(warm_ps, ones, ones, start=True, stop=True)

            x_tiles = [None] * NG
            pend = [None] * NG  # g -> (q_ps3, kv_ps, xT)

            def emit_xdma(g):
                x_nat = pa.tile([P, HID], F32R, tag="xnat", bufs=3)
                nc.sync.dma_start(out=x_nat, in_=x_d[g * P : (g + 1) * P, :])
                x_tiles[g] = x_nat

            # DMA order: x0, wq(2), wkv, x1, cos, sin, x2.., wo(4) trailing
            emit_xdma(0)
            nc.sync.dma_start(out=wq_sb[:, 0:4, :], in_=wq_r[:, 0:4, :])
            nc.sync.dma_start(out=wq_sb[:, 4:8, :], in_=wq_r[:, 4:8, :])
            emit_xdma(1)
            nc.sync.dma_start(out=wq_sb[:, 8:12, :], in_=wq_r[:, 8:12, :])
            nc.sync.dma_start(out=wq_sb[:, 12:16, :], in_=wq_r[:, 12:16, :])
            nc.sync.dma_start(
                out=wkv_sb[:, :, 0:D], in_=wk_d[:].rearrange("(c p) n -> p c n", p=P)
            )
            nc.sync.dma_start(
                out=wkv_sb[:, :, D : 2 * D],
                in_=wv_d[:].rearrange("(c p) n -> p c n", p=P),
            )
            emit_xdma(2)
            nc.sync.dma_start(
                out=cos_sb, in_=cos_d[:].rearrange("(c p) d -> p c d", p=P)
            )
            nc.sync.dma_start(
                out=sin_sb, in_=sin_d[:].rearrange("(c p) d -> p c d", p=P)
            )
            wo_next = [0]

            def emit_wo_dma():
                h = wo_next[0]
                if h < NH:
                    nc.sync.dma_start(out=wo_sb[:, h, :], in_=wo_r[:, h, :])
                    wo_next[0] += 1

            def transposes(g):
                """x chunk -> xT (PE transpose + DVE cast-copy)."""
                x_nat = x_tiles[g]
                xT = pa.tile([P, KC, P], F32R, tag="xT", bufs=3)
                xT_flat = xT.rearrange("p c d -> p (c d)")
                for kb in range(KC // 4):
                    tp_ps = ps_tp.tile([P, 4 * P], F32R, tag="tp")
                    for j in range(4):
                        k = 4 * kb + j
                        nc.tensor.transpose(
                            tp_ps[:, j * P : (j + 1) * P],
                            x_nat[:, k * P : (k + 1) * P],
                            ident,
                        )
                    if kb % 2 == 0:
                        nc.vector.tensor_copy(
                            xT_flat[:, kb * 4 * P : (kb + 1) * 4 * P], tp_ps
                        )
                    else:
                        nc.scalar.activation(
                            out=xT_flat[:, kb * 4 * P : (kb + 1) * 4 * P], in_=tp_ps,
                            func=AF.Copy,
                        )
                return xT

            def proj(g, xT):
                """q and kv projections for chunk g (PE, accumulating)."""
                qkv_ps = ps_qkv.tile([P, NH * D + 2 * D], F32, tag="qkv")
                q_ps = qkv_ps[:, 0 : NH * D]
                kv_ps = qkv_ps[:, NH * D : NH * D + 2 * D]
                for k in range(KC):
                    nc.tensor.matmul(
                        q_ps, xT[:, k, :], wq_sb[:, k, :],
                        start=(k == 0), stop=(k == KC - 1),
                    )
                for k in range(KC):
                    nc.tensor.matmul(
                        kv_ps, xT[:, k, :], wkv_sb[:, k, :],
                        start=(k == 0), stop=(k == KC - 1),
                    )
                return qkv_ps

            def rope_stage(g, qkv_ps):
                """RoPE on q heads + k (DVE), v copy-out."""
                q3 = qkv_ps[:, 0 : NH * D].rearrange("p (h d) -> p h d", h=NH)
                kv_ps = qkv_ps[:, NH * D : NH * D + 2 * D]
                q_rope = pa.tile([P, NH, D], F32R, tag="qrope")
                for h in range(NH):
                    rope(q_rope[:, h, :], q3[:, h, :], g, "tmq", pa)
                k_rope = pa.tile([P, D], F32R, tag="krope")
                rope(k_rope, kv_ps[:, 0:D], g, "tmk", pa)
                nc.vector.tensor_copy(vv[:, g, :], kv_ps[:, D : 2 * D])
                return q_rope, k_rope

            def rope_transpose(g, q_rope, k_rope):
                """Transpose RoPE'd q/k into persistent qT_all / kT."""
                tq_ps = ps_tp.tile([P, 4 * P], F32R, tag="tp")
                for h in range(NH):
                    nc.tensor.transpose(
                        tq_ps[:, h * P : (h + 1) * P], q_rope[:, h, :], ident
                    )
                nc.vector.tensor_copy(
                    qT_all[:, :, g * P : (g + 1) * P],
                    tq_ps.rearrange("p (h d) -> p h d", h=NH),
                )
                tk_ps = ps_tp.tile([P, 4 * P], F32R, tag="tp")
                nc.tensor.transpose(tk_ps[:, 0:P], k_rope, ident)
                nc.vector.tensor_copy(kT[:, g * P : (g + 1) * P], tk_ps[:, 0:P])

            # 2-deep software pipeline over chunks
            ropes = [None] * NG
            for g in range(NG + 2):
                if g >= 2:
                    gg = g - 2
                    sc = nc.named_scope(f"rope_{gg}"); sc.__enter__()
                    ropes[gg] = rope_stage(gg, pend[gg][1])
                    sc.__exit__(None, None, None)
                if g < NG:
                    if g + 3 < NG:
                        emit_xdma(g + 3)
                    if g >= 4:
                        emit_wo_dma()
                    sc = nc.named_scope(f"tp_{g}"); sc.__enter__()
                    xT = transposes(g)
                    sc.__exit__(None, None, None)
                    pend[g] = [xT, None, None]
                if g >= 1 and g - 1 < NG:
                    gg = g - 1
                    sc = nc.named_scope(f"proj_{gg}"); sc.__enter__()
                    qkv_ps = proj(gg, pend[gg][0])
                    sc.__exit__(None, None, None)
                    pend[gg][1] = qkv_ps
                if g >= 2:
                    gg = g - 2
                    sc = nc.named_scope(f"ropeT_{gg}"); sc.__enter__()
                    rope_transpose(gg, *ropes[gg])
                    sc.__exit__(None, None, None)
                    pend[gg] = None
            emit_wo_dma()
            emit_wo_dma()
            emit_wo_dma()
            emit_wo_dma()

        # ================= Phase B/C: attention + output =================
        with tc.tile_pool(name="pB_sb", bufs=2) as pb, \
             tc.tile_pool(name="pB_s", bufs=3, space="PSUM") as ps_s, \
             tc.tile_pool(name="pB_ud", bufs=2, space="PSUM") as ps_ud, \
             tc.tile_pool(name="pB_y", bufs=2, space="PSUM") as ps_y:

            def scores_head(t, h):
                """scoresT + exp for head h of macro tile t -> expst tile.

                Chunk pairs share one full PSUM bank so the causal mask is a
                single DVE add and exp is one ACT op per pair."""
                qT_h = qT_all[:, h, t * TQ : (t + 1) * TQ]
                expst = pb.tile([P, NSK, TQ], F32R, tag="expst")
                expst_flat = expst.rearrange("p c f -> p (c f)")
                for pi in range(t + 1):
                    s_ps = ps_s.tile([P, 2 * TQ], F32, tag="s")
                    for half in range(2):
                        ik = 2 * pi + half
                        nc.tensor.matmul(
                            s_ps[:, half * TQ : (half + 1) * TQ],
                            kT[:, ik * P : (ik + 1) * P], qT_h,
                            start=True, stop=True,
                        )
                    if pi == t:
                        nc.vector.tensor_tensor(out=s_ps, in0=s_ps, in1=m12, op=AL.add)
                    nc.scalar.activation(
                        out=expst_flat[:, pi * 2 * TQ : (pi + 1) * 2 * TQ],
                        in_=s_ps, func=AF.Exp, scale=SCALE,
                    )
                return expst

            def dnpv_head(t, h, expst, uT_t):
                """denominator + PV matmuls, then normalize into uT_t (DVE)."""
                nsk = 2 * (t + 1)
                ud_ps = ps_ud.tile([P, 2 * TQ], F32, tag="ud")
                u_ps = ud_ps[:, 0:TQ]
                den_ps = ud_ps[:, TQ : 2 * TQ]
                for ik in range(nsk):
                    nc.tensor.matmul(
                        den_ps, ones, expst[:, ik, :],
                        start=(ik == 0), stop=(ik == nsk - 1),
                    )
                for ik in range(nsk):
                    nc.tensor.matmul(
                        u_ps, vv[:, ik, :], expst[:, ik, :],
                        start=(ik == 0), stop=(ik == nsk - 1),
                    )
                rec = pb.tile([P, TQ], F32, tag="rec")
                nc.vector.reciprocal(rec, den_ps)
                nc.vector.tensor_tensor(
                    out=uT_t[:, h, :], in0=u_ps, in1=rec, op=AL.mult
                )

            def wo_stage(t, uT_t):
                for sub in range(2):
                    g = 2 * t + sub
                    for n in range(HID // 512):
                        y_ps = ps_y.tile([P, 512], F32, tag="y")
                        for h in range(NH):
                            nc.tensor.matmul(
                                y_ps,
                                uT_t[:, h, sub * P : (sub + 1) * P],
                                wo_sb[:, h, n * 512 : (n + 1) * 512],
                                start=(h == 0), stop=(h == NH - 1),
                            )
                        y_sb = pb.tile([P, 512], F32, tag="ysb", bufs=3)
                        nc.vector.tensor_copy(y_sb, y_ps)
                        nc.gpsimd.dma_start(
                            out=out_d[g * P : (g + 1) * P, n * 512 : (n + 1) * 512],
                            in_=y_sb,
                        )

            # heads pipelined one deep; wo lags two score-steps so the
            # last head's DVE normalize is off the PE critical path
            steps = [(t, h) for t in reversed(range(NT)) for h in range(NH)]
            uts = {}
            for i in range(len(steps) + 2):
                if i < len(steps):
                    t, h = steps[i]
                    if h == 0:
                        uts[t] = pb.tile([P, NH, TQ], F32R, tag="uT", name=f"uT{t}")
                    sc = nc.named_scope(f"sc_{t}_{h}"); sc.__enter__()
                    uts[(t, h)] = scores_head(t, h)
                    sc.__exit__(None, None, None)
                if 1 <= i < len(steps) + 1:
                    t, h = steps[i - 1]
                    sc = nc.named_scope(f"dnpv_{t}_{h}"); sc.__enter__()
                    dnpv_head(t, h, uts.pop((t, h)), uts[t])
                    sc.__exit__(None, None, None)
                if i >= 2 and (i - 2) % NH == NH - 1:
                    t = steps[i - 2][0]
                    sc = nc.named_scope(f"wo_{t}"); sc.__enter__()
                    wo_stage(t, uts.pop(t))
                    sc.__exit__(None, None, None)

    nc.compile()
    return nc


def shard_inputs(x, cos, sin, wq, wk, wv, wo):
    """Build per-core input maps: core = b*4 + g."""
    in_maps = []
    for c in range(N_CORES):
        b, g = divmod(c, N_KV)
        in_maps.append(
            {
                "x": np.ascontiguousarray(x[b]),
                "cos": np.ascontiguousarray(cos),
                "sin": np.ascontiguousarray(sin),
                "wq": np.ascontiguousarray(wq[:, g * NH * D : (g + 1) * NH * D]),
                "wk": np.ascontiguousarray(wk[:, g * D : (g + 1) * D]),
                "wv": np.ascontiguousarray(wv[:, g * D : (g + 1) * D]),
                "wo": np.ascontiguousarray(wo[g * NH * D : (g + 1) * NH * D, :]),
            }
        )
    return in_maps


_NC_CACHE = {}


def get_nc():
    if "nc" not in _NC_CACHE:
        _NC_CACHE["nc"] = build_nc()
    return _NC_CACHE["nc"]


def kernel(x, cos, sin, wq, wk, wv, wo, _trace=False):
    from concourse.bass_utils import run_bass_kernel_spmd

    x = np.asarray(x, dtype=np.float32)
    cos = np.asarray(cos, dtype=np.float32)
    sin = np.asarray(sin, dtype=np.float32)
    wq = np.asarray(wq, dtype=np.float32)
    wk = np.asarray(wk, dtype=np.float32)
    wv = np.asarray(wv, dtype=np.float32)
    wo = np.asarray(wo, dtype=np.float32)

    nc = get_nc()
    in_maps = shard_inputs(x, cos, sin, wq, wk, wv, wo)
    res = run_bass_kernel_spmd(nc, in_maps, list(range(N_CORES)), trace=_trace)
    parts = [np.asarray(res.results[c]["out"], dtype=np.float32) for c in range(N_CORES)]
    y = np.stack(
        [sum(parts[b * N_KV + g] for g in range(N_KV)) for b in range(B)], axis=0
    )
    if _trace:
        kernel.last_result = res
    return y
